# revision 16
# baseline (speedup 1.0000x reference)
"""Trainium2 Bass kernel for nn_GATNet (5-net dual-branch GAT + dense tail).

Decomposition (validated against the reference with numpy prototypes):
  - 10 independent GAT "branches" = (x1|x2) x 5 nets. Core c owns branch c
    fully (c=0..7); branches 8,9 are split into dst-quarters across cores
    0-3 / 4-7 (20 of 80 dst-windows each).
  - Attention softmax max-subtraction cancels algebraically -> plain exp.
  - al/ar attention dot-products fold into the input matmul weights.
  - Edge aggregation: per 128-dst window, gather [h|al] rows of an H table
    by src (dma_gather), build one-hot S via iota==ldst, segment-sum on the
    PE: acc[d, f] += S.T @ (pe * h_src), plus a 10-col matmul for the denom.
  - dst-side ar values come from window-local rows expanded per-edge with a
    host-precomputed one-hot-transpose matmul (no dst gather). The per-edge
    weighting is split: 6 heads on Vector, 4 on Scalar (Copy with scale).
  - Layer 2 (single head): H2 rows [v|1|al2|ar2]; attention weight folded
    into S2 = onehot*pe so one 129-col matmul yields numerator + denom.
  - Max-pool via padded index gather + strided reduce_max; Wg + 1/5-mean
    combine via AllReduce(add); dense tail runs (redundantly) on all cores.
Cross-core: AllGather of quarter-branch H2, AllReduce-max of quarter pooled,
AllReduce-add of branch contributions. Collectives sit between TileContexts
with explicit completion semaphores and Pool-engine fences; readers of
collective outputs go through gpsimd DMAs (Pool FIFO ordering).
"""

import numpy as np
import ml_dtypes

BF16 = ml_dtypes.bfloat16
F8NP = ml_dtypes.float8_e4m3    # one-hot tables: 0/1 exact in fp8

NUM_NET = 5
N = 10000
E = 160000
B = 512
FXD = 62
FXT = 954
D = 128
H1 = 10
NCORE = 8
NWIN = 80            # 80 windows x 128 dst rows = 10240
QWIN = 20
HC = H1 * D          # 1280
HROW = 1408          # H table row: [h(1280) | al(10) | pad]  (bf16 2816B)
H2ROW = 256          # H2/OUT2 row: [v(128) | al2 | ar2 | pad] (bf16 512B)
NEG = -3.0e38
CH = 6               # edge tiles per gather chunk
NQ = 4               # SWDGE queues (desc-gen parallelism across Q7 core pairs)
NH_ACT = 2           # heads of the L1 per-edge weighting done on Scalar engine


def _wrap_idx(idx):
    """dma_gather index layout: [128, ceil(n/16)] int16; idx i -> [i%16, i//16], x8 replicated."""
    n = len(idx)
    ncol = (n + 15) // 16
    a = np.zeros((16, ncol), np.int16)
    m = np.arange(n)
    a[m % 16, m // 16] = idx.astype(np.int16)
    return np.tile(a, (8, 1))


def _sort_edges(ei):
    src = np.concatenate([ei[0], np.arange(N, dtype=np.int64)])
    dst = np.concatenate([ei[1], np.arange(N, dtype=np.int64)])
    order = np.argsort(dst, kind="stable")
    return src[order], dst[order]


def _pack_part(src_s, dst_s, bounds, windows, T):
    esrc_cols, ldst_cols, sT_cols, sS_cols = [], [], [], []
    jj = np.arange(128)
    for slot, w in enumerate(windows):
        s, e = bounds[w], bounds[w + 1]
        npad = T[slot] * 128
        srcp = np.zeros(npad, np.int64)
        srcp[: e - s] = src_s[s:e]
        ldp = np.full(npad, -1.0, np.float32)
        ldp[: e - s] = (dst_s[s:e] - w * 128).astype(np.float32)
        esrc_cols.append(_wrap_idx(srcp))
        ldst_cols.append(ldp.reshape(T[slot], 128).T.copy())
        ld = ldp.reshape(T[slot], 128)
        # one-hot transpose per tile: sT[j, t*128+p] = (ldst[t,p] == j)
        st = (jj[None, :, None] == ld[:, None, :])  # [T, 128j, 128p]
        sT_cols.append(
            st.transpose(1, 0, 2).reshape(128, npad).astype(F8NP))
        # scatter one-hot per tile: sS[p, t*128+j] = (ldst[t,p] == j)
        ss = (ld[:, :, None] == jj[None, None, :])  # [T, 128p, 128j]
        sS_cols.append(
            ss.transpose(1, 0, 2).reshape(128, npad).astype(F8NP))
    return (
        np.concatenate(esrc_cols, axis=1),
        np.concatenate(sT_cols, axis=1),
        np.concatenate(sS_cols, axis=1),
        np.concatenate(ldst_cols, axis=1),
    )


def _pool_idx(batch, slot, row_lo, row_hi, row_off, sentinel):
    counts = np.bincount(batch, minlength=B)
    starts = np.zeros(B + 1, np.int64)
    np.cumsum(counts, out=starts[1:])
    tbl = np.full((B, slot), sentinel, np.int64)
    for g in range(B):
        nodes = np.arange(starts[g], starts[g + 1])
        nodes = nodes[(nodes >= row_lo) & (nodes < row_hi)]
        if len(nodes) == 0:
            continue
        loc = nodes - row_off
        k = min(len(loc), slot)
        tbl[g, :k] = loc[:k]
        if k < slot:
            tbl[g, k:] = loc[k - 1]
    flat = tbl.T.reshape(-1)  # j = s*512 + g
    return _wrap_idx(flat)


def _sched(parts_sel, max_win=None):
    """Window emission order: interleave f and q 4:1 so every engine FIFO
    always holds independent work from the other stream."""
    out = []
    if tuple(parts_sel) == ("f", "q"):
        qi = 0
        for w in range(NWIN):
            out.append(("f", w))
            if w % 4 == 3 and qi < QWIN:
                out.append(("q", qi))
                qi += 1
    else:
        for part in parts_sel:
            out += [(part, w) for w in
                    range(NWIN if part == "f" else QWIN)]
    if max_win is not None:
        out = [(pt, w) for pt, w in out if w < max_win]
    return out


def _fold_weights(W, a_s, a_d):
    Hh, C = a_s.shape
    F = W.shape[1]
    Wr = W.reshape(Hh, C, F)
    return np.einsum("hcf,hc->hf", Wr, a_s), np.einsum("hcf,hc->hf", Wr, a_d)


def _prep_inputs(inputs):
    g = {k: np.asarray(v) for k, v in inputs.items()}
    branches = []
    for b in range(10):
        net = b % 5
        if b < 5:
            x, ei, batch = g["x1"][net], g["ei1"][net], g["batch1"][net]
        else:
            x, ei, batch = g["x2"][net], g["ei2"][net], g["batch2"][net]
        src_s, dst_s = _sort_edges(ei)
        bounds = np.searchsorted(dst_s, np.arange(NWIN + 1) * 128)
        branches.append(dict(net=net, x=x, batch=batch, src=src_s, dst=dst_s,
                             bounds=bounds))

    T_full = np.ones(NWIN, np.int64)
    for c in range(NCORE):
        br = branches[c]
        cnt = np.maximum(br["bounds"][1:] - br["bounds"][:-1], 1)
        T_full = np.maximum(T_full, (cnt + 127) // 128)
    T_qtr = np.ones(QWIN, np.int64)
    for c in range(NCORE):
        qb = 8 if c < 4 else 9
        q = c % 4
        br = branches[qb]
        w0 = q * QWIN
        cnt = br["bounds"][w0 + 1:w0 + QWIN + 1] - br["bounds"][w0:w0 + QWIN]
        cnt = np.maximum(cnt, 1)
        T_qtr = np.maximum(T_qtr, (cnt + 127) // 128)

    slot = 1
    for b in range(10):
        slot = max(slot, int(np.bincount(branches[b]["batch"], minlength=B).max()))

    # head-interleaved feature order for layer-1: col (d*H1 + h) <- h*D + d.
    # Makes the per-edge alpha-weighting broadcast have a real unit-stride
    # innermost dim (10 contiguous heads) so the DVE runs in 2x mode.
    ILP = (np.arange(H1)[None, :] * D + np.arange(D)[:, None]).reshape(-1)

    def branch_weights(b):
        net = b % 5
        W1, a1s, a1d, b1 = g["W1"][net], g["a1s"][net], g["a1d"][net], g["b1"][net]
        W2, a2s, a2d, b2 = g["W2"][net], g["a2s"][net], g["a2d"][net], g["b2"][net]
        Wg, bg = g["Wg"][net], g["bg"][net]
        Was, Wad = _fold_weights(W1, a1s, a1d)
        w1ext = np.ascontiguousarray(
            np.concatenate([W1[ILP], Was, Wad], 0).T).astype(BF16)  # [62, 1300]
        W2s, W2d = _fold_weights(W2, a2s, a2d)
        w2extT = np.concatenate([W2, W2s, W2d], 0).T[ILP]       # [1280, 130]
        w2pack = np.zeros((128, 1300), BF16)
        for c in range(10):
            w2pack[:, 130 * c:130 * (c + 1)] = w2extT[128 * c:128 * (c + 1), :]
        b1rep = np.tile(b1[ILP][None, :], (128, 1)).astype(BF16)
        b2rep = np.tile(b2[None, :], (128, 1)).astype(BF16)
        wgT = np.ascontiguousarray(Wg.T).astype(BF16)           # [128,128]
        bgc = np.ascontiguousarray(bg[:, None]).astype(np.float32)
        return w1ext, w2pack, b1rep, b2rep, wgT, bgc

    wr1T = np.zeros((1024, 2048), np.float32)
    wr1T[:FXT, :] = g["Wr1"].T
    tail = dict(
        cell=np.ascontiguousarray(g["cell"]).astype(np.float32),
        wr1T=wr1T,
        br1c=np.ascontiguousarray(g["br1"].reshape(16, 128).T).astype(np.float32),
        wr2T=np.ascontiguousarray(g["Wr2"].T).astype(np.float32),
        br2c=np.ascontiguousarray(g["br2"].reshape(4, 128).T).astype(np.float32),
        wr3T=np.ascontiguousarray(g["Wr3"].T).astype(np.float32),
        br3c=np.ascontiguousarray(g["br3"].reshape(2, 128).T).astype(np.float32),
        wf1T=np.ascontiguousarray(g["Wf1"].T).astype(np.float32),
        bf1c=np.ascontiguousarray(g["bf1"].reshape(16, 128).T).astype(np.float32),
        wf2T=np.ascontiguousarray(g["Wf2"].T).astype(np.float32),
        bf2c=np.ascontiguousarray(g["bf2"].reshape(4, 128).T).astype(np.float32),
        wf3T=np.ascontiguousarray(g["Wf3"].T).astype(np.float32),
        bf3c=np.ascontiguousarray(g["bf3"].reshape(1, 128).T).astype(np.float32),
        woT=np.ascontiguousarray(g["Wo"].T).astype(np.float32),
        boc=np.ascontiguousarray(g["bo"][:, None]).astype(np.float32),
    )

    in_maps = []
    for c in range(NCORE):
        m = {}
        for part, bidx, windows, T in (
            ("f", c, list(range(NWIN)), T_full),
            ("q", 8 if c < 4 else 9,
             list(range((c % 4) * QWIN, (c % 4) * QWIN + QWIN)), T_qtr),
        ):
            br = branches[bidx]
            xT = np.zeros((FXD, NWIN * 128), BF16)
            xT[:, :N] = br["x"].T.astype(BF16)
            esrc, sT, sS, ldst = _pack_part(br["src"], br["dst"],
                                            br["bounds"], windows, T)
            w1ext, w2pack, b1rep, b2rep, wgT, bgc = branch_weights(bidx)
            if part == "f":
                pidx = _pool_idx(br["batch"], slot, 0, N, 0, NWIN * 128)
            else:
                lo = (c % 4) * QWIN * 128
                pidx = _pool_idx(br["batch"], slot, lo, min(lo + QWIN * 128, N),
                                 lo, QWIN * 128)
            wsc = np.zeros((128, 2), np.float32)
            if part == "f":
                wsc[:, 0 if bidx < 5 else 1] = 0.2
            elif c in (0, 4):
                wsc[:, 1] = 0.2   # branches 8,9 counted once per group
            if part == "q":
                lo = (c % 4) * QWIN * 128
                m["qwidx_q"] = _wrap_idx(np.arange(lo, lo + QWIN * 128))
            m.update({
                f"xT_{part}": xT, f"w1ext_{part}": w1ext,
                f"w2pack_{part}": w2pack, f"b1rep_{part}": b1rep,
                f"b2rep_{part}": b2rep, f"wgT_{part}": wgT,
                f"bgc_{part}": bgc, f"wsc_{part}": wsc,
                f"esrc_{part}": esrc, f"sT_{part}": sT, f"sS_{part}": sS,
                f"ldst_{part}": ldst, f"pidx_{part}": pidx,
            })
        m.update(tail)
        in_maps.append(m)

    consts = dict(T_full=list(map(int, T_full)), T_qtr=list(map(int, T_qtr)),
                  slot=slot)
    return in_maps, consts


# ---------------------------------------------------------------------------


def _build(consts, stage=4, parts_sel=("f", "q"), dbg=False, max_win=None):
    import concourse.bacc as bacc
    import concourse.mybir as mybir
    import concourse.tile as tile
    from concourse.tile_rust import add_dep_helper
    from concourse.masks import make_identity

    F32 = mybir.dt.float32
    F8 = mybir.dt.float8e4
    BF = mybir.dt.bfloat16
    I16 = mybir.dt.int16
    ALU = mybir.AluOpType
    ACTF = mybir.ActivationFunctionType
    AX = mybir.AxisListType

    T_full = consts["T_full"]
    T_qtr = consts["T_qtr"]
    SLOT = consts["slot"]

    nc = bacc.Bacc("TRN2", target_bir_lowering=False, debug=False,
                   num_devices=NCORE, num_swdge_queues=NQ)
    qctr = [0]

    def nextq():
        q = qctr[0] % NQ
        qctr[0] += 1
        return q

    def ein(name, shape, dt):
        return nc.dram_tensor(name, shape, dt, kind="ExternalInput").ap()

    parts = {}
    for part, T, nw in (("f", T_full, NWIN), ("q", T_qtr, QWIN)):
        sT = int(np.sum(T))
        p = dict(
            part=part, T=T, nw=nw,
            cumT=np.concatenate([[0], np.cumsum(T)]).astype(int),
            xT=ein(f"xT_{part}", [FXD, NWIN * 128], BF),
            w1ext=ein(f"w1ext_{part}", [FXD, 1300], BF),
            w2pack=ein(f"w2pack_{part}", [128, 1300], BF),
            b1rep=ein(f"b1rep_{part}", [128, HC], BF),
            b2rep=ein(f"b2rep_{part}", [128, D], BF),
            wgT=ein(f"wgT_{part}", [128, 128], BF),
            bgc=ein(f"bgc_{part}", [128, 1], F32),
            wsc=ein(f"wsc_{part}", [128, 2], F32),
            esrc=ein(f"esrc_{part}", [128, 8 * sT], I16),
            stbl=ein(f"sT_{part}", [128, 128 * sT], F8),
            sstbl=ein(f"sS_{part}", [128, 128 * sT], F8),
            ldst=ein(f"ldst_{part}", [128, sT], F32),
            pidx=ein(f"pidx_{part}", [128, (B * SLOT) // 16], I16),
        )
        if part == "q":
            p["qwidx"] = ein("qwidx_q", [128, (QWIN * 128) // 16], I16)
        nrow = NWIN * 128 if part == "f" else QWIN * 128
        kind = "ExternalOutput" if (dbg and stage < 3) else "Internal"
        kind_h2 = "Internal" if (part == "q" and stage >= 2) else kind
        p["H"] = nc.dram_tensor(f"Htbl_{part}", [NWIN * 128, HROW], BF, kind=kind).ap()
        p["ADt"] = nc.dram_tensor(f"ADtbl_{part}", [NWIN * 128, 128], BF, kind=kind).ap()
        p["H2w"] = nc.dram_tensor(f"H2w_{part}", [nrow, H2ROW], BF, kind=kind_h2).ap()
        out_rows = nrow + 128
        p["OUT2"] = nc.dram_tensor(f"OUT2_{part}", [out_rows, H2ROW], BF, kind=kind).ap()
        p["sentinel"] = out_rows - 128
        parts[part] = p
    H2q_full = nc.dram_tensor("H2q_full", [NWIN * 128, H2ROW], BF).ap()
    parts["f"]["H2read"] = parts["f"]["H2w"]
    parts["q"]["H2read"] = H2q_full

    kindd = "ExternalOutput" if (dbg and stage < 3) else "Internal"
    pooledq_in = nc.dram_tensor("pooledq_in", [128, 1024], F32, kind=kindd).ap()
    pooledq_red = nc.dram_tensor("pooledq_red", [128, 1024], F32).ap()
    C_accum = nc.dram_tensor("C_accum", [256, 512], F32, kind=kindd).ap()
    C_in = nc.dram_tensor("C_in", [256, 512], F32).ap()
    C_red = nc.dram_tensor("C_red", [256, 512], F32).ap()

    cell_in = ein("cell", [B, FXT], F32)
    tw = {k: ein(k, shp, F32) for k, shp in (
        ("wr1T", [1024, 2048]), ("br1c", [128, 16]),
        ("wr2T", [2048, 512]), ("br2c", [128, 4]),
        ("wr3T", [512, 256]), ("br3c", [128, 2]),
        ("wf1T", [512, 2048]), ("bf1c", [128, 16]),
        ("wf2T", [2048, 512]), ("bf2c", [128, 4]),
        ("wf3T", [512, 128]), ("bf3c", [128, 1]),
        ("woT", [128, 2]), ("boc", [2, 1]),
    )}
    out_ext = nc.dram_tensor("out", [2, B], F32, kind="ExternalOutput").ap()

    sem1 = nc.semaphore("ccs1").__enter__()
    sem2 = nc.semaphore("ccs2").__enter__()
    sem3 = nc.semaphore("ccs3").__enter__()

    def chunks_of(T):
        out, t0 = [], 0
        while t0 < T:
            out.append((t0, min(CH, T - t0)))
            t0 += CH
        return out

    # =======================================================================
    # TC1: phase A (H/AD tables) + GAT1 windows + mm2 -> H2 tables
    # =======================================================================
    with tile.TileContext(nc) as tc:
        with tc.tile_pool(name="const1", bufs=1) as cst, \
             tc.tile_pool(name="work1", bufs=2) as wp, \
             tc.tile_pool(name="mwork", bufs=3) as mp, \
             tc.tile_pool(name="ps1", bufs=2, space="PSUM") as ps, \
             tc.tile_pool(name="psA1", bufs=1, space="PSUM") as psA, \
             tc.tile_pool(name="ps_t", bufs=1, space="PSUM") as pst:

            idenb = cst.tile([128, 128], BF)
            make_identity(nc, idenb)

            h2_writes = []
            pctx = {}
            for part in parts_sel:
                p = parts[part]
                w1sb = cst.tile([FXD, 1300], BF, tag=f"w1sb{part}")
                nc.sync.dma_start(w1sb, p["w1ext"])
                w2sb = cst.tile([128, 1300], BF, tag=f"w2sb{part}")
                nc.sync.dma_start(w2sb, p["w2pack"])
                b1sb = cst.tile([128, HC], BF, tag=f"b1sb{part}")
                nc.sync.dma_start(b1sb, p["b1rep"])

                # ---- phase A ----
                arall = cst.tile([128, NWIN, H1], BF, tag=f"arall{part}")
                for c in range(NWIN):
                    xT = wp.tile([FXD, 128], BF, tag="xT")
                    nc.sync.dma_start(xT, p["xT"][:, 128 * c:128 * (c + 1)])
                    acc = ps.tile([128, 1300], F32, tag="acc",
                                  padded_shape=[128, 1536])
                    for n0, n1 in ((0, 512), (512, 1024), (1024, 1300)):
                        nc.tensor.matmul(acc[:, n0:n1], lhsT=xT,
                                         rhs=w1sb[:, n0:n1],
                                         start=True, stop=True)
                    hrow = wp.tile([128, HROW], BF, tag="hrow")
                    if dbg:
                        nc.vector.memset(hrow[:, 1290:HROW], 0.0)
                    nc.scalar.activation(hrow[:, 0:1290], acc[:, 0:1290],
                                         ACTF.Copy)
                    adrow = wp.tile([128, 128], BF, tag="adrow")
                    if dbg:
                        nc.vector.memset(adrow[:, 10:128], 0.0)
                    nc.vector.tensor_copy(adrow[:, 0:10], acc[:, 1290:1300])
                    nc.vector.tensor_copy(arall[:, c, :], acc[:, 1290:1300])
                    nc.sync.dma_start(p["H"][128 * c:128 * (c + 1), :], hrow)
                    nc.sync.dma_start(p["ADt"][128 * c:128 * (c + 1), :], adrow)

                arq = None
                if part == "q":
                    # q's dst windows are a core-dependent global slice of
                    # ADt; fetch their ar rows via one host-indexed gather.
                    qwi = cst.tile([128, (QWIN * 128) // 16], I16, tag="qwi")
                    nc.sync.dma_start(qwi, p["qwidx"])
                    arq = cst.tile([128, QWIN, 128], BF, tag="arq")
                    nc.gpsimd.dma_gather(arq, p["ADt"], qwi, QWIN * 128,
                                         QWIN * 128, 128, single_packet=False,
                                         queue_num=nextq())
                pctx[part] = (w1sb, w2sb, b1sb, arall, arq)

            # ---- GAT1 windows, f/q interleaved 4:1 to fill pipeline ----
            for part, w in _sched(parts_sel, max_win):
                    p = parts[part]
                    w1sb, w2sb, b1sb, arall, arq = pctx[part]
                    T = p["T"][w]
                    c0 = int(p["cumT"][w])
                    acc = ps.tile([128, 1426], F32, tag="acc",
                                  padded_shape=[128, 1536])
                    arl = arall[:, w, :] if part == "f" else arq[:, w, 0:10]
                    for t0, nt in chunks_of(T):
                        cc = c0 + t0
                        sidx = wp.tile([128, 8 * nt], I16, tag="sidx", bufs=4)
                        nc.sync.dma_start(sidx,
                                          p["esrc"][:, 8 * cc:8 * (cc + nt)])
                        G = wp.tile([128, nt, HROW], BF, tag="G", bufs=4)
                        nc.gpsimd.dma_gather(G, p["H"], sidx, nt * 128,
                                             nt * 128, HROW, single_packet=False,
                                             queue_num=nextq())
                        sTc = wp.tile([128, nt, 128], F8, tag="sTc", bufs=3)
                        nc.sync.dma_start(
                            sTc, p["stbl"][:, 128 * cc:128 * (cc + nt)])
                        sSc = wp.tile([128, nt, 128], F8, tag="sSc", bufs=3)
                        nc.sync.dma_start(
                            sSc, p["sstbl"][:, 128 * cc:128 * (cc + nt)])
                        adps = psA.tile([128, nt, H1], F32, tag="adps")
                        for tl in range(nt):
                            nc.tensor.matmul(adps[:, tl, :],
                                             lhsT=sTc[:, tl, :], rhs=arl,
                                             start=True, stop=True)
                        z = wp.tile([128, nt, H1], F32, tag="z")
                        nc.vector.tensor_tensor(z, G[:, :, 1280:1290],
                                                adps, op=ALU.add)
                        z2 = wp.tile([128, nt, H1], F32, tag="z2")
                        nc.vector.tensor_scalar(z2, z, 0.2, None, op0=ALU.mult)
                        nc.vector.tensor_tensor(z, z, z2, op=ALU.max)
                        pe = wp.tile([128, nt, H1], BF, tag="pe")
                        nc.scalar.activation(pe, z, ACTF.Exp)
                        # head-interleaved layout: pe broadcast has a real
                        # unit-stride innermost dim of 10 -> DVE 2x mode
                        Mc = wp.tile([128, nt, 1290], BF, tag="Mc")
                        nc.vector.tensor_tensor(
                            Mc[:, :, 0:1280].rearrange(
                                "p t (d h) -> p t d h", h=H1),
                            G[:, :, 0:1280].rearrange(
                                "p t (d h) -> p t d h", h=H1),
                            pe.rearrange("p t (o h) -> p t o h", o=1)
                              .to_broadcast([128, nt, D, H1]),
                            op=ALU.mult)
                        nc.vector.tensor_copy(Mc[:, :, 1280:1290], pe)
                        for tl in range(nt):
                            t = t0 + tl
                            for n0, n1 in ((0, 512), (512, 1024), (1024, 1290)):
                                nc.tensor.matmul(acc[:, n0:n1],
                                                 lhsT=sSc[:, tl, :],
                                                 rhs=Mc[:, tl, n0:n1],
                                                 start=(t == 0),
                                                 stop=(t == T - 1))
                    # window tail
                    rden = wp.tile([128, H1, 1], F32, tag="rden")
                    nc.vector.tensor_scalar(rden[:, :, 0], acc[:, 1280:1290],
                                            1e-30, None, op0=ALU.add)
                    nc.vector.reciprocal(rden[:, :, 0], rden[:, :, 0])
                    z1 = wp.tile([128, HC], BF, tag="z1")
                    nc.vector.tensor_tensor(
                        z1.rearrange("p (d h) -> p d h", h=H1),
                        acc[:, 0:1280].rearrange("p (d h) -> p d h", h=H1),
                        rden.rearrange("p h o -> p o h")
                            .to_broadcast([128, D, H1]),
                        op=ALU.mult)
                    nc.vector.tensor_tensor(z1, z1, b1sb, op=ALU.add)
                    e1 = wp.tile([128, HC], BF, tag="e1")
                    nc.vector.tensor_scalar(e1, z1, 0.0, None, op0=ALU.min)
                    nc.scalar.activation(e1, e1, ACTF.Exp)
                    r1 = wp.tile([128, HC], BF, tag="r1")
                    nc.scalar.activation(r1, z1, ACTF.Relu)
                    nc.vector.tensor_tensor(r1, r1, e1, op=ALU.add)
                    nc.vector.tensor_scalar(r1, r1, -1.0, None, op0=ALU.add)
                    # xbar-transposes + mm2 -> h2 (accumulate in acc's pad)
                    h2ps = acc[:, 1296:1426]
                    for c in range(10):
                        o1T = mp.tile([128, 128], BF, tag="o1T")
                        nc.scalar.dma_start_transpose(
                            o1T, r1[:, 128 * c:128 * (c + 1)])
                        nc.tensor.matmul(h2ps, lhsT=o1T,
                                         rhs=w2sb[:, 130 * c:130 * (c + 1)],
                                         start=(c == 0), stop=(c == 9))
                    h2sb = wp.tile([128, H2ROW], BF, tag="h2sb")
                    if dbg:
                        nc.vector.memset(h2sb[:, 131:H2ROW], 0.0)
                    nc.vector.tensor_copy(h2sb[:, 0:128], h2ps[:, 0:128])
                    nc.vector.memset(h2sb[:, 128:129], 1.0)
                    nc.vector.tensor_copy(h2sb[:, 129:131], h2ps[:, 128:130])
                    wrh2 = nc.sync.dma_start(
                        p["H2w"][128 * w:128 * (w + 1), :], h2sb)
                    h2_writes.append(wrh2)

            flag = cst.tile([1, 1], F32)
            fence = nc.gpsimd.memset(flag, 0.0)
            for wrh2 in h2_writes:
                add_dep_helper(fence.ins, wrh2.ins, sync=True,
                               reason="fence H2 writes")

    if stage < 2:
        nc.compile()
        return nc
    if "q" in parts_sel:
        nc.gpsimd.collective_compute(
            "AllGather", mybir.AluOpType.bypass,
            replica_groups=[[0, 1, 2, 3], [4, 5, 6, 7]],
            ins=[parts["q"]["H2w"]], outs=[H2q_full],
        ).then_inc(sem1)
        nc.gpsimd.wait_ge(sem1, 1)

    # =======================================================================
    # TC2: layer-2 windows + pooling; full-branch Wg + contributions
    # =======================================================================
    with tile.TileContext(nc) as tc:
        with tc.tile_pool(name="const2", bufs=1) as cst, \
             tc.tile_pool(name="work2", bufs=2) as wp, \
             tc.tile_pool(name="mwork2", bufs=3) as mp, \
             tc.tile_pool(name="ps2", bufs=2, space="PSUM") as ps, \
             tc.tile_pool(name="psA2", bufs=1, space="PSUM") as psA, \
             tc.tile_pool(name="ps2t", bufs=2, space="PSUM") as pst:

            idenf = cst.tile([128, 128], F32, tag="idenf")
            make_identity(nc, idenf)
            x11T = cst.tile([128, 512], F32, tag="x11T")
            nc.vector.memset(x11T, 0.0)
            x22T = cst.tile([128, 512], F32, tag="x22T")
            nc.vector.memset(x22T, 0.0)

            fence_deps = []
            pctx2 = {}
            for part in parts_sel:
                p = parts[part]
                b2sb = cst.tile([128, D], BF, tag=f"b2sb{part}")
                nc.sync.dma_start(b2sb, p["b2rep"])
                sent = cst.tile([1, H2ROW], BF, tag=f"sent{part}")
                nc.vector.memset(sent, NEG)
                nc.sync.dma_start(
                    p["OUT2"][p["sentinel"]:p["sentinel"] + 1, :], sent)
                pctx2[part] = b2sb

            # ---- GAT2 windows, f/q interleaved ----
            for part, w in _sched(parts_sel, max_win):
                    p = parts[part]
                    b2sb = pctx2[part]
                    T = p["T"][w]
                    c0 = int(p["cumT"][w])
                    ar2l = wp.tile([128, 1], BF, tag="ar2l")
                    nc.sync.dma_start(
                        ar2l, p["H2w"][128 * w:128 * (w + 1), 130:131])
                    acc2 = ps.tile([128, 129], F32, tag="acc2")
                    for t0, nt in chunks_of(T):
                        cc = c0 + t0
                        sidx = wp.tile([128, 8 * nt], I16, tag="sidx", bufs=4)
                        nc.sync.dma_start(sidx,
                                          p["esrc"][:, 8 * cc:8 * (cc + nt)])
                        G2 = wp.tile([128, nt, H2ROW], BF, tag="G2", bufs=4)
                        nc.gpsimd.dma_gather(G2, p["H2read"], sidx, nt * 128,
                                             nt * 128, H2ROW, single_packet=False,
                                             queue_num=nextq())
                        sTc = wp.tile([128, nt, 128], F8, tag="sTc", bufs=3)
                        nc.sync.dma_start(
                            sTc, p["stbl"][:, 128 * cc:128 * (cc + nt)])
                        sSc = wp.tile([128, nt, 128], F8, tag="sSc", bufs=3)
                        nc.sync.dma_start(
                            sSc, p["sstbl"][:, 128 * cc:128 * (cc + nt)])
                        ar2ps = psA.tile([128, nt, 1], F32, tag="ar2ps")
                        for tl in range(nt):
                            nc.tensor.matmul(ar2ps[:, tl, :],
                                             lhsT=sTc[:, tl, :], rhs=ar2l,
                                             start=True, stop=True)
                        z = wp.tile([128, nt, 1], F32, tag="z")
                        nc.vector.tensor_tensor(z, G2[:, :, 129:130],
                                                ar2ps, op=ALU.add)
                        z2 = wp.tile([128, nt, 1], F32, tag="z2")
                        nc.vector.tensor_scalar(z2, z, 0.2, None, op0=ALU.mult)
                        nc.vector.tensor_tensor(z, z, z2, op=ALU.max)
                        pe = wp.tile([128, nt, 1], F32, tag="pe")
                        nc.scalar.activation(pe, z, ACTF.Exp)
                        S2c = wp.tile([128, nt, 128], BF, tag="S2c")
                        nc.vector.tensor_tensor(
                            S2c, sSc, pe.to_broadcast([128, nt, 128]),
                            op=ALU.mult)
                        for tl in range(nt):
                            t = t0 + tl
                            nc.tensor.matmul(acc2[:, 0:129], lhsT=S2c[:, tl, :],
                                             rhs=G2[:, tl, 0:129],
                                             start=(t == 0), stop=(t == T - 1))
                    rden = wp.tile([128, 1], F32, tag="rden")
                    nc.vector.tensor_scalar(rden, acc2[:, 128:129], 1e-30,
                                            None, op0=ALU.add)
                    nc.vector.reciprocal(rden, rden)
                    z1 = wp.tile([128, D], BF, tag="z1")
                    nc.vector.tensor_tensor(z1, acc2[:, 0:128],
                                            rden.to_broadcast([128, D]),
                                            op=ALU.mult)
                    nc.vector.tensor_tensor(z1, z1, b2sb, op=ALU.add)
                    e1 = wp.tile([128, D], BF, tag="e1")
                    nc.vector.tensor_scalar(e1, z1, 0.0, None, op0=ALU.min)
                    nc.scalar.activation(e1, e1, ACTF.Exp)
                    o2 = wp.tile([128, H2ROW], BF, tag="o2")
                    if dbg:
                        nc.vector.memset(o2[:, 128:H2ROW], 0.0)
                    nc.scalar.activation(o2[:, 0:128], z1, ACTF.Relu)
                    nc.vector.tensor_tensor(o2[:, 0:128], o2[:, 0:128], e1,
                                            op=ALU.add)
                    nc.vector.tensor_scalar(o2[:, 0:128], o2[:, 0:128], -1.0,
                                            None, op0=ALU.add)
                    nc.sync.dma_start(p["OUT2"][128 * w:128 * (w + 1), :], o2)

            for part in parts_sel:
                p = parts[part]
                # ---- pooling ----
                pidx = wp.tile([128, (B * SLOT) // 16], I16, tag="pidx")
                nc.sync.dma_start(pidx, p["pidx"])
                Pg = wp.tile([128, SLOT * 4, H2ROW], BF, tag="Pg", bufs=1)
                # split pooling gather 4 ways across SWDGE queues
                npq = (B * SLOT) // NQ
                assert npq % 128 == 0 and (SLOT * 4) % NQ == 0
                for qq in range(NQ):
                    d1 = (SLOT * 4) // NQ
                    nc.gpsimd.dma_gather(
                        Pg[:, d1 * qq:d1 * (qq + 1), :], p["OUT2"],
                        pidx[:, (npq // 16) * qq:(npq // 16) * (qq + 1)],
                        npq, npq, H2ROW, single_packet=False, queue_num=qq)
                pooled = wp.tile([128, 4, H2ROW], F32, tag="pooled", bufs=1)
                nc.vector.tensor_reduce(
                    pooled,
                    Pg.rearrange("p (s gb) e -> p gb e s", s=SLOT),
                    op=ALU.max, axis=AX.X)

                if part == "f":
                    wg = cst.tile([128, 128], BF, tag="wg")
                    nc.sync.dma_start(wg, p["wgT"])
                    bgc = cst.tile([128, 1], F32, tag="bgcs")
                    nc.sync.dma_start(bgc, p["bgc"])
                    wsc = cst.tile([128, 2], F32, tag="wscs")
                    nc.sync.dma_start(wsc, p["wsc"])
                    gps = ps.tile([128, 512], F32, tag="gps")
                    for gb in range(4):
                        tp = pst.tile([128, 128], F32, tag="tp2")
                        nc.tensor.transpose(tp, pooled[:, gb, 0:128], idenf)
                        pT = mp.tile([128, 128], BF, tag="pT")
                        nc.vector.tensor_copy(pT, tp)
                        nc.tensor.matmul(gps[:, 128 * gb:128 * (gb + 1)],
                                         lhsT=wg, rhs=pT,
                                         start=True, stop=True)
                    gt = wp.tile([128, 512], F32, tag="gt", bufs=1)
                    nc.vector.tensor_scalar(gt, gps, bgc, 0.0, op0=ALU.add,
                                            op1=ALU.max)
                    c11 = wp.tile([128, 512], F32, tag="c11", bufs=1)
                    nc.vector.tensor_scalar(c11, gt, wsc[:, 0:1], None,
                                            op0=ALU.mult)
                    nc.vector.tensor_tensor(x11T, x11T, c11, op=ALU.add)
                    nc.vector.tensor_scalar(c11, gt, wsc[:, 1:2], None,
                                            op0=ALU.mult)
                    nc.vector.tensor_tensor(x22T, x22T, c11, op=ALU.add)
                else:
                    wrp = nc.sync.dma_start(
                        pooledq_in, pooled.rearrange("p gb e -> p (gb e)"))
                    fence_deps.append(wrp)

            wc1 = nc.sync.dma_start(C_accum[0:128, :], x11T)
            wc2 = nc.sync.dma_start(C_accum[128:256, :], x22T)
            fence_deps += [wc1, wc2]
            flag = cst.tile([1, 1], F32, tag="flag2")
            fence = nc.gpsimd.memset(flag, 0.0)
            for dpi in fence_deps:
                add_dep_helper(fence.ins, dpi.ins, sync=True, reason="fence2")

    if stage < 3:
        nc.compile()
        return nc
    nc.gpsimd.collective_compute(
        "AllReduce", mybir.AluOpType.max,
        replica_groups=[[0, 1, 2, 3], [4, 5, 6, 7]],
        ins=[pooledq_in], outs=[pooledq_red],
    ).then_inc(sem2)
    nc.gpsimd.wait_ge(sem2, 1)

    # =======================================================================
    # TC3: quarter-branch Wg + add contribution
    # =======================================================================
    with tile.TileContext(nc) as tc:
        with tc.tile_pool(name="const3", bufs=1) as cst, \
             tc.tile_pool(name="work3", bufs=2) as wp, \
             tc.tile_pool(name="ps3", bufs=1, space="PSUM") as ps, \
             tc.tile_pool(name="ps3t", bufs=2, space="PSUM") as pst:
            p = parts["q"]
            idenf = cst.tile([128, 128], F32)
            make_identity(nc, idenf)
            pq = cst.tile([128, 1024], F32)
            nc.gpsimd.dma_start(pq, pooledq_red)
            ca = cst.tile([128, 512], F32)
            nc.gpsimd.dma_start(ca, C_accum[0:128, :])
            cb = cst.tile([128, 512], F32)
            nc.gpsimd.dma_start(cb, C_accum[128:256, :])
            wg = cst.tile([128, 128], BF)
            nc.sync.dma_start(wg, p["wgT"])
            bgc = cst.tile([128, 1], F32)
            nc.sync.dma_start(bgc, p["bgc"])
            wsc = cst.tile([128, 2], F32)
            nc.sync.dma_start(wsc, p["wsc"])
            pooled = pq.rearrange("p (gb e) -> p gb e", gb=4)
            gps = ps.tile([128, 512], F32)
            for gb in range(4):
                tp = pst.tile([128, 128], F32, tag="tp3")
                nc.tensor.transpose(tp, pooled[:, gb, 0:128], idenf)
                pT = wp.tile([128, 128], BF, tag="pT3")
                nc.vector.tensor_copy(pT, tp)
                nc.tensor.matmul(gps[:, 128 * gb:128 * (gb + 1)], lhsT=wg,
                                 rhs=pT, start=True, stop=True)
            gt = cst.tile([128, 512], F32, tag="gt3")
            nc.vector.tensor_scalar(gt, gps, bgc, 0.0, op0=ALU.add,
                                    op1=ALU.max)
            cq = cst.tile([128, 512], F32, tag="cq")
            nc.vector.tensor_scalar(cq, gt, wsc[:, 0:1], None, op0=ALU.mult)
            nc.vector.tensor_tensor(ca, ca, cq, op=ALU.add)
            nc.vector.tensor_scalar(cq, gt, wsc[:, 1:2], None, op0=ALU.mult)
            nc.vector.tensor_tensor(cb, cb, cq, op=ALU.add)
            w1 = nc.sync.dma_start(C_in[0:128, :], ca)
            w2 = nc.sync.dma_start(C_in[128:256, :], cb)
            flag = cst.tile([1, 1], F32, tag="flag3")
            fence = nc.gpsimd.memset(flag, 0.0)
            add_dep_helper(fence.ins, w1.ins, sync=True, reason="fence3a")
            add_dep_helper(fence.ins, w2.ins, sync=True, reason="fence3b")

    nc.gpsimd.collective_compute(
        "AllReduce", mybir.AluOpType.add,
        replica_groups=[list(range(NCORE))],
        ins=[C_in], outs=[C_red],
    ).then_inc(sem3)
    nc.gpsimd.wait_ge(sem3, 1)

    # =======================================================================
    # TC4: dense tail (full batch on every core)
    # =======================================================================
    with tile.TileContext(nc) as tc:
        with tc.tile_pool(name="const4", bufs=1) as cst, \
             tc.tile_pool(name="work4", bufs=2) as wp, \
             tc.tile_pool(name="twp", bufs=2) as twp, \
             tc.tile_pool(name="ps4", bufs=2, space="PSUM") as ps, \
             tc.tile_pool(name="ps4t", bufs=2, space="PSUM") as pst:

            idenf = cst.tile([128, 128], F32)
            make_identity(nc, idenf)
            x11T = cst.tile([128, 512], F32)
            nc.gpsimd.dma_start(x11T, C_red[0:128, :])
            x22T = cst.tile([128, 512], F32)
            nc.gpsimd.dma_start(x22T, C_red[128:256, :])

            cn_chunks = []
            for r in range(4):
                cr = wp.tile([128, FXT], F32, tag="cellr")
                nc.sync.dma_start(cr, cell_in[128 * r:128 * (r + 1), :])
                sq = wp.tile([128, FXT], F32, tag="sqc")
                nc.vector.tensor_tensor(sq, cr, cr, op=ALU.mult)
                rn = wp.tile([128, 1], F32, tag="rn")
                nc.vector.tensor_reduce(rn, sq, op=ALU.add, axis=AX.X)
                nc.scalar.activation(rn, rn, ACTF.Sqrt)
                nc.vector.reciprocal(rn, rn)
                cn = cst.tile([128, 1024], F32, tag=f"cn{r}")
                nc.vector.tensor_scalar(cn[:, 0:FXT], cr, rn, None,
                                        op0=ALU.mult)
                nc.vector.memset(cn[:, FXT:1024], 0.0)
                cn_chunks.append(cn)
            cT = []
            for k in range(8):
                ct = cst.tile([128, 512], F32, tag=f"cT{k}")
                for r in range(4):
                    tp = pst.tile([128, 128], F32, tag="tp4")
                    nc.tensor.transpose(
                        tp, cn_chunks[r][:, 128 * k:128 * (k + 1)], idenf)
                    nc.vector.tensor_copy(ct[:, 128 * r:128 * (r + 1)], tp)
                cT.append(ct)

            def dense(chunks_in, win, bin_, kdim, mdim, tagp):
                nk = kdim // 128
                nm = mdim // 128
                assert len(chunks_in) == nk
                bsb = cst.tile([128, nm], F32, tag=f"b{tagp}")
                nc.sync.dma_start(bsb, bin_)
                outs = []
                for m in range(nm):
                    wsb = twp.tile([128, nk, 128], F32, tag="tw")
                    nc.sync.dma_start(
                        wsb,
                        win.rearrange("(k p) m -> p k m", p=128)
                           [:, :, 128 * m:128 * (m + 1)])
                    acc = ps.tile([128, 512], F32, tag="acc4")
                    for k in range(nk):
                        nc.tensor.matmul(acc, lhsT=wsb[:, k, :],
                                         rhs=chunks_in[k],
                                         start=(k == 0), stop=(k == nk - 1))
                    ot = cst.tile([128, 512], F32, tag=f"o{tagp}{m}")
                    nc.vector.tensor_scalar(ot, acc, bsb[:, m:m + 1], 0.0,
                                            op0=ALU.add, op1=ALU.max)
                    outs.append(ot)
                return outs

            r1 = dense(cT, tw["wr1T"], tw["br1c"], 1024, 2048, "r1")
            r2 = dense(r1, tw["wr2T"], tw["br2c"], 2048, 512, "r2")
            r3 = dense(r2, tw["wr3T"], tw["br3c"], 512, 256, "r3")

            xc = [x11T, x22T, r3[0], r3[1]]
            ones = cst.tile([128, 1], F32, tag="ones")
            nc.vector.memset(ones, 1.0)
            n2ps = ps.tile([1, 512], F32, tag="n2ps", bufs=1)
            for i, chk in enumerate(xc):
                sq = wp.tile([128, 512], F32, tag="sq")
                nc.vector.tensor_tensor(sq, chk, chk, op=ALU.mult)
                nc.tensor.matmul(n2ps, lhsT=ones, rhs=sq,
                                 start=(i == 0), stop=(i == 3))
            nrm = wp.tile([1, 512], F32, tag="nrm4")
            nc.scalar.activation(nrm, n2ps, ACTF.Sqrt)
            rn4 = wp.tile([1, 512], F32, tag="rn4")
            nc.vector.reciprocal(rn4, nrm)
            rnb = cst.tile([128, 512], F32, tag="rnb")
            nc.gpsimd.partition_broadcast(rnb, rn4)
            xcn = []
            for i, chk in enumerate(xc):
                o = cst.tile([128, 512], F32, tag=f"xcn{i}")
                nc.vector.tensor_tensor(o, chk, rnb, op=ALU.mult)
                xcn.append(o)

            f1 = dense(xcn, tw["wf1T"], tw["bf1c"], 512, 2048, "f1")
            f2 = dense(f1, tw["wf2T"], tw["bf2c"], 2048, 512, "f2")
            f3 = dense(f2, tw["wf3T"], tw["bf3c"], 512, 128, "f3")

            wo = cst.tile([128, 2], F32, tag="wo")
            nc.sync.dma_start(wo, tw["woT"])
            bo = cst.tile([2, 1], F32, tag="bo")
            nc.sync.dma_start(bo, tw["boc"])
            ops_ = ps.tile([2, 512], F32, tag="ops", bufs=1)
            nc.tensor.matmul(ops_, lhsT=wo, rhs=f3[0], start=True, stop=True)
            osb = cst.tile([2, 512], F32, tag="osb")
            nc.vector.tensor_scalar(osb, ops_, bo, None, op0=ALU.add)
            nc.sync.dma_start(out_ext, osb)

    nc.compile()
    return nc


_CACHE = {}


def kernel(**inputs) -> np.ndarray:
    from concourse.bass_utils import run_bass_kernel_spmd

    in_maps, consts = _prep_inputs(inputs)
    key = (tuple(consts["T_full"]), tuple(consts["T_qtr"]), consts["slot"])
    if key not in _CACHE:
        _CACHE[key] = _build(consts)
    nc = _CACHE[key]
    res = run_bass_kernel_spmd(nc, in_maps, core_ids=list(range(NCORE)))
    return np.ascontiguousarray(
        np.asarray(res.results[0]["out"]).T).astype(np.float32)



# revision 25
# speedup vs baseline: 1.2938x; 1.2938x over previous
"""Trainium2 Bass kernel for nn_GATNet (5-net dual-branch GAT + dense tail).

Decomposition (validated against the reference with numpy prototypes):
  - 10 independent GAT "branches" = (x1|x2) x 5 nets. Core c owns branch c
    fully (c=0..7); branches 8,9 are split into dst-quarters across cores
    0-3 / 4-7 (20 of 80 dst-windows each).
  - Attention softmax max-subtraction cancels algebraically -> plain exp.
  - al/ar attention dot-products fold into the input matmul weights.
  - Edge aggregation: per 128-dst window, gather [h|al] rows of an H table
    by src (dma_gather), build one-hot S via iota==ldst, segment-sum on the
    PE: acc[d, f] += S.T @ (pe * h_src), plus a 10-col matmul for the denom.
  - dst-side ar values come from window-local rows expanded per-edge with a
    host-precomputed one-hot-transpose matmul (no dst gather). The per-edge
    weighting is split: 6 heads on Vector, 4 on Scalar (Copy with scale).
  - Layer 2 (single head): H2 rows [v|1|al2|ar2]; attention weight folded
    into S2 = onehot*pe so one 129-col matmul yields numerator + denom.
  - Max-pool via padded index gather + strided reduce_max; Wg + 1/5-mean
    combine via AllReduce(add); dense tail runs (redundantly) on all cores.
Cross-core: AllGather of quarter-branch H2, AllReduce-max of quarter pooled,
AllReduce-add of branch contributions. Collectives sit between TileContexts
with explicit completion semaphores and Pool-engine fences; readers of
collective outputs go through gpsimd DMAs (Pool FIFO ordering).
"""

import numpy as np
import ml_dtypes

BF16 = ml_dtypes.bfloat16
F8NP = ml_dtypes.float8_e4m3    # one-hot tables: 0/1 exact in fp8

NUM_NET = 5
N = 10000
E = 160000
B = 512
FXD = 62
FXT = 954
D = 128
H1 = 10
NCORE = 8
NWIN = 80            # 80 windows x 128 dst rows = 10240
QWIN = 20
HC = H1 * D          # 1280
HROW = 1408          # H table row: [h(1280) | al(10) | pad]  (bf16 2816B)
H2ROW = 256          # H2/OUT2 row: [v(128) | al2 | ar2 | pad] (bf16 512B)
NEG = -3.0e38
CH = 6               # edge tiles per gather chunk
NQ = 4               # SWDGE queues (desc-gen parallelism across Q7 core pairs)
NH_ACT = 2           # heads of the L1 per-edge weighting done on Scalar engine


def _wrap_idx(idx):
    """dma_gather index layout: [128, ceil(n/16)] int16; idx i -> [i%16, i//16], x8 replicated."""
    n = len(idx)
    ncol = (n + 15) // 16
    a = np.zeros((16, ncol), np.int16)
    m = np.arange(n)
    a[m % 16, m // 16] = idx.astype(np.int16)
    return np.tile(a, (8, 1))


def _sort_edges(ei, with_self):
    """Sorted-by-dst edge list; self-loops appended only when requested
    (the f-part handles them as a dedicated identity tile instead)."""
    if with_self:
        src = np.concatenate([ei[0], np.arange(N, dtype=np.int64)])
        dst = np.concatenate([ei[1], np.arange(N, dtype=np.int64)])
    else:
        src = ei[0].astype(np.int64)
        dst = ei[1].astype(np.int64)
    order = np.argsort(dst, kind="stable")
    return src[order], dst[order]


def _pack_part(src_s, dst_s, bounds, windows, T):
    esrc_cols, ldst_cols, sT_cols, sS_cols = [], [], [], []
    jj = np.arange(128)
    for slot, w in enumerate(windows):
        s, e = bounds[w], bounds[w + 1]
        npad = T[slot] * 128
        srcp = np.zeros(npad, np.int64)
        srcp[: e - s] = src_s[s:e]
        ldp = np.full(npad, -1.0, np.float32)
        ldp[: e - s] = (dst_s[s:e] - w * 128).astype(np.float32)
        esrc_cols.append(_wrap_idx(srcp))
        ldst_cols.append(ldp.reshape(T[slot], 128).T.copy())
        ld = ldp.reshape(T[slot], 128)
        # one-hot transpose per tile: sT[j, t*128+p] = (ldst[t,p] == j)
        st = (jj[None, :, None] == ld[:, None, :])  # [T, 128j, 128p]
        sT_cols.append(
            st.transpose(1, 0, 2).reshape(128, npad).astype(F8NP))
        # scatter one-hot per tile: sS[p, t*128+j] = (ldst[t,p] == j)
        ss = (ld[:, :, None] == jj[None, None, :])  # [T, 128p, 128j]
        sS_cols.append(
            ss.transpose(1, 0, 2).reshape(128, npad).astype(F8NP))
    return (
        np.concatenate(esrc_cols, axis=1),
        np.concatenate(sT_cols, axis=1),
        np.concatenate(sS_cols, axis=1),
        np.concatenate(ldst_cols, axis=1),
    )


def _pool_idx(batch, slot, row_lo, row_hi, row_off, sentinel):
    counts = np.bincount(batch, minlength=B)
    starts = np.zeros(B + 1, np.int64)
    np.cumsum(counts, out=starts[1:])
    tbl = np.full((B, slot), sentinel, np.int64)
    for g in range(B):
        nodes = np.arange(starts[g], starts[g + 1])
        nodes = nodes[(nodes >= row_lo) & (nodes < row_hi)]
        if len(nodes) == 0:
            continue
        loc = nodes - row_off
        k = min(len(loc), slot)
        tbl[g, :k] = loc[:k]
        if k < slot:
            tbl[g, k:] = loc[k - 1]
    flat = tbl.T.reshape(-1)  # j = s*512 + g
    return _wrap_idx(flat)


def _sched(parts_sel, max_win=None):
    """Window emission order: interleave f and q 4:1 so every engine FIFO
    always holds independent work from the other stream."""
    out = []
    if tuple(parts_sel) == ("f", "q"):
        qi = 0
        for w in range(NWIN):
            out.append(("f", w))
            if w % 4 == 3 and qi < QWIN:
                out.append(("q", qi))
                qi += 1
    else:
        for part in parts_sel:
            out += [(part, w) for w in
                    range(NWIN if part == "f" else QWIN)]
    if max_win is not None:
        out = [(pt, w) for pt, w in out if w < max_win]
    return out


def _fold_weights(W, a_s, a_d):
    Hh, C = a_s.shape
    F = W.shape[1]
    Wr = W.reshape(Hh, C, F)
    return np.einsum("hcf,hc->hf", Wr, a_s), np.einsum("hcf,hc->hf", Wr, a_d)


def _prep_inputs(inputs):
    g = {k: np.asarray(v) for k, v in inputs.items()}
    branches = []
    for b in range(10):
        net = b % 5
        if b < 5:
            x, ei, batch = g["x1"][net], g["ei1"][net], g["batch1"][net]
        else:
            x, ei, batch = g["x2"][net], g["ei2"][net], g["batch2"][net]
        # f-part: no self-loops in the stream (dedicated identity tile);
        # q-part keeps them inline.
        src_ns, dst_ns = _sort_edges(ei, with_self=False)
        bounds_ns = np.searchsorted(dst_ns, np.arange(NWIN + 1) * 128)
        src_s, dst_s = _sort_edges(ei, with_self=True)
        bounds = np.searchsorted(dst_s, np.arange(NWIN + 1) * 128)
        branches.append(dict(net=net, x=x, batch=batch, src=src_s, dst=dst_s,
                             bounds=bounds, src_ns=src_ns, dst_ns=dst_ns,
                             bounds_ns=bounds_ns))

    T_full = np.ones(NWIN, np.int64)
    for c in range(NCORE):
        br = branches[c]
        cnt = np.maximum(br["bounds_ns"][1:] - br["bounds_ns"][:-1], 1)
        T_full = np.maximum(T_full, (cnt + 127) // 128)
    T_qtr = np.ones(QWIN, np.int64)
    for c in range(NCORE):
        qb = 8 if c < 4 else 9
        q = c % 4
        br = branches[qb]
        w0 = q * QWIN
        cnt = br["bounds"][w0 + 1:w0 + QWIN + 1] - br["bounds"][w0:w0 + QWIN]
        cnt = np.maximum(cnt, 1)
        T_qtr = np.maximum(T_qtr, (cnt + 127) // 128)

    slot = 1
    for b in range(10):
        slot = max(slot, int(np.bincount(branches[b]["batch"], minlength=B).max()))

    # head-interleaved feature order for layer-1: col (d*H1 + h) <- h*D + d.
    # Makes the per-edge alpha-weighting broadcast have a real unit-stride
    # innermost dim (10 contiguous heads) so the DVE runs in 2x mode.
    ILP = (np.arange(H1)[None, :] * D + np.arange(D)[:, None]).reshape(-1)

    def branch_weights(b):
        net = b % 5
        W1, a1s, a1d, b1 = g["W1"][net], g["a1s"][net], g["a1d"][net], g["b1"][net]
        W2, a2s, a2d, b2 = g["W2"][net], g["a2s"][net], g["a2d"][net], g["b2"][net]
        Wg, bg = g["Wg"][net], g["bg"][net]
        Was, Wad = _fold_weights(W1, a1s, a1d)
        w1ext = np.ascontiguousarray(
            np.concatenate([W1[ILP], Was, Wad], 0).T).astype(BF16)  # [62, 1300]
        W2s, W2d = _fold_weights(W2, a2s, a2d)
        w2extT = np.concatenate([W2, W2s, W2d], 0).T[ILP]       # [1280, 130]
        w2pack = np.zeros((128, 1300), BF16)
        for c in range(10):
            w2pack[:, 130 * c:130 * (c + 1)] = w2extT[128 * c:128 * (c + 1), :]
        b1rep = np.tile(b1[ILP][None, :], (128, 1)).astype(BF16)
        b2rep = np.tile(b2[None, :], (128, 1)).astype(BF16)
        wgT = np.ascontiguousarray(Wg.T).astype(BF16)           # [128,128]
        bgc = np.ascontiguousarray(bg[:, None]).astype(np.float32)
        return w1ext, w2pack, b1rep, b2rep, wgT, bgc

    wr1T = np.zeros((1024, 2048), np.float32)
    wr1T[:FXT, :] = g["Wr1"].T
    tail = dict(
        cell=np.ascontiguousarray(g["cell"]).astype(np.float32),
        wr1T=wr1T,
        br1c=np.ascontiguousarray(g["br1"].reshape(16, 128).T).astype(np.float32),
        wr2T=np.ascontiguousarray(g["Wr2"].T).astype(np.float32),
        br2c=np.ascontiguousarray(g["br2"].reshape(4, 128).T).astype(np.float32),
        wr3T=np.ascontiguousarray(g["Wr3"].T).astype(np.float32),
        br3c=np.ascontiguousarray(g["br3"].reshape(2, 128).T).astype(np.float32),
        wf1T=np.ascontiguousarray(g["Wf1"].T).astype(np.float32),
        bf1c=np.ascontiguousarray(g["bf1"].reshape(16, 128).T).astype(np.float32),
        wf2T=np.ascontiguousarray(g["Wf2"].T).astype(np.float32),
        bf2c=np.ascontiguousarray(g["bf2"].reshape(4, 128).T).astype(np.float32),
        wf3T=np.ascontiguousarray(g["Wf3"].T).astype(np.float32),
        bf3c=np.ascontiguousarray(g["bf3"].reshape(1, 128).T).astype(np.float32),
        woT=np.ascontiguousarray(g["Wo"].T).astype(np.float32),
        boc=np.ascontiguousarray(g["bo"][:, None]).astype(np.float32),
    )

    in_maps = []
    for c in range(NCORE):
        m = {}
        for part, bidx, windows, T in (
            ("f", c, list(range(NWIN)), T_full),
            ("q", 8 if c < 4 else 9,
             list(range((c % 4) * QWIN, (c % 4) * QWIN + QWIN)), T_qtr),
        ):
            br = branches[bidx]
            xT = np.zeros((FXD, NWIN * 128), BF16)
            xT[:, :N] = br["x"].T.astype(BF16)
            if part == "f":
                esrc, sT, sS, ldst = _pack_part(br["src_ns"], br["dst_ns"],
                                                br["bounds_ns"], windows, T)
            else:
                esrc, sT, sS, ldst = _pack_part(br["src"], br["dst"],
                                                br["bounds"], windows, T)
            w1ext, w2pack, b1rep, b2rep, wgT, bgc = branch_weights(bidx)
            if part == "f":
                pidx = _pool_idx(br["batch"], slot, 0, N, 0, NWIN * 128)
            else:
                lo = (c % 4) * QWIN * 128
                pidx = _pool_idx(br["batch"], slot, lo, min(lo + QWIN * 128, N),
                                 lo, QWIN * 128)
            wsc = np.zeros((128, 2), np.float32)
            if part == "f":
                wsc[:, 0 if bidx < 5 else 1] = 0.2
            elif c in (0, 4):
                wsc[:, 1] = 0.2   # branches 8,9 counted once per group
            if part == "q":
                lo = (c % 4) * QWIN * 128
                m["qwidx_q"] = _wrap_idx(np.arange(lo, lo + QWIN * 128))
            m.update({
                f"xT_{part}": xT, f"w1ext_{part}": w1ext,
                f"w2pack_{part}": w2pack, f"b1rep_{part}": b1rep,
                f"b2rep_{part}": b2rep, f"wgT_{part}": wgT,
                f"bgc_{part}": bgc, f"wsc_{part}": wsc,
                f"esrc_{part}": esrc, f"sT_{part}": sT, f"sS_{part}": sS,
                f"ldst_{part}": ldst, f"pidx_{part}": pidx,
            })
        m.update(tail)
        in_maps.append(m)

    consts = dict(T_full=list(map(int, T_full)), T_qtr=list(map(int, T_qtr)),
                  slot=slot)
    return in_maps, consts


# ---------------------------------------------------------------------------


def _build(consts, stage=4, parts_sel=("f", "q"), dbg=False, max_win=None):
    import concourse.bacc as bacc
    import concourse.mybir as mybir
    import concourse.tile as tile
    from concourse.tile_rust import add_dep_helper
    from concourse.masks import make_identity

    F32 = mybir.dt.float32
    F8 = mybir.dt.float8e4
    BF = mybir.dt.bfloat16
    I16 = mybir.dt.int16
    ALU = mybir.AluOpType
    ACTF = mybir.ActivationFunctionType
    AX = mybir.AxisListType

    T_full = consts["T_full"]
    T_qtr = consts["T_qtr"]
    SLOT = consts["slot"]

    nc = bacc.Bacc("TRN2", target_bir_lowering=False, debug=False,
                   num_devices=NCORE, num_swdge_queues=NQ)
    qctr = [0]

    def nextq():
        q = qctr[0] % NQ
        qctr[0] += 1
        return q

    def ein(name, shape, dt):
        return nc.dram_tensor(name, shape, dt, kind="ExternalInput").ap()

    parts = {}
    for part, T, nw in (("f", T_full, NWIN), ("q", T_qtr, QWIN)):
        sT = int(np.sum(T))
        p = dict(
            part=part, T=T, nw=nw,
            cumT=np.concatenate([[0], np.cumsum(T)]).astype(int),
            xT=ein(f"xT_{part}", [FXD, NWIN * 128], BF),
            w1ext=ein(f"w1ext_{part}", [FXD, 1300], BF),
            w2pack=ein(f"w2pack_{part}", [128, 1300], BF),
            b1rep=ein(f"b1rep_{part}", [128, HC], BF),
            b2rep=ein(f"b2rep_{part}", [128, D], BF),
            wgT=ein(f"wgT_{part}", [128, 128], BF),
            bgc=ein(f"bgc_{part}", [128, 1], F32),
            wsc=ein(f"wsc_{part}", [128, 2], F32),
            esrc=ein(f"esrc_{part}", [128, 8 * sT], I16),
            stbl=ein(f"sT_{part}", [128, 128 * sT], F8),
            sstbl=ein(f"sS_{part}", [128, 128 * sT], F8),
            ldst=ein(f"ldst_{part}", [128, sT], F32),
            pidx=ein(f"pidx_{part}", [128, (B * SLOT) // 16], I16),
        )
        if part == "q":
            p["qwidx"] = ein("qwidx_q", [128, (QWIN * 128) // 16], I16)
        nrow = NWIN * 128 if part == "f" else QWIN * 128
        kind = "ExternalOutput" if (dbg and stage < 3) else "Internal"
        kind_h2 = "Internal" if (part == "q" and stage >= 2) else kind
        p["H"] = nc.dram_tensor(f"Htbl_{part}", [NWIN * 128, HROW], BF, kind=kind).ap()
        p["ADt"] = nc.dram_tensor(f"ADtbl_{part}", [NWIN * 128, 128], BF, kind=kind).ap()
        p["H2w"] = nc.dram_tensor(f"H2w_{part}", [nrow, H2ROW], BF, kind=kind_h2).ap()
        out_rows = nrow + 128
        p["OUT2"] = nc.dram_tensor(f"OUT2_{part}", [out_rows, H2ROW], BF, kind=kind).ap()
        p["sentinel"] = out_rows - 128
        parts[part] = p
    H2q_full = nc.dram_tensor("H2q_full", [NWIN * 128, H2ROW], BF).ap()
    parts["f"]["H2read"] = parts["f"]["H2w"]
    parts["q"]["H2read"] = H2q_full

    kindd = "ExternalOutput" if (dbg and stage < 3) else "Internal"
    pooledq_in = nc.dram_tensor("pooledq_in", [128, 1024], F32, kind=kindd).ap()
    pooledq_red = nc.dram_tensor("pooledq_red", [128, 1024], F32).ap()
    C_accum = nc.dram_tensor("C_accum", [256, 512], F32, kind=kindd).ap()
    C_in = nc.dram_tensor("C_in", [256, 512], F32).ap()
    C_red = nc.dram_tensor("C_red", [256, 512], F32).ap()

    cell_in = ein("cell", [B, FXT], F32)
    tw = {k: ein(k, shp, F32) for k, shp in (
        ("wr1T", [1024, 2048]), ("br1c", [128, 16]),
        ("wr2T", [2048, 512]), ("br2c", [128, 4]),
        ("wr3T", [512, 256]), ("br3c", [128, 2]),
        ("wf1T", [512, 2048]), ("bf1c", [128, 16]),
        ("wf2T", [2048, 512]), ("bf2c", [128, 4]),
        ("wf3T", [512, 128]), ("bf3c", [128, 1]),
        ("woT", [128, 2]), ("boc", [2, 1]),
    )}
    out_ext = nc.dram_tensor("out", [2, B], F32, kind="ExternalOutput").ap()

    sem1 = nc.semaphore("ccs1").__enter__()
    sem2 = nc.semaphore("ccs2").__enter__()
    sem3 = nc.semaphore("ccs3").__enter__()

    def chunks_of(T):
        out, t0 = [], 0
        while t0 < T:
            out.append((t0, min(CH, T - t0)))
            t0 += CH
        return out

    # =======================================================================
    # TC1: phase A (H/AD tables) + GAT1 windows + mm2 -> H2 tables
    # =======================================================================
    with tile.TileContext(nc) as tc:
        with tc.tile_pool(name="const1", bufs=1) as cst, \
             tc.tile_pool(name="work1", bufs=2) as wp, \
             tc.tile_pool(name="mwork", bufs=3) as mp, \
             tc.tile_pool(name="ps1", bufs=2, space="PSUM") as ps, \
             tc.tile_pool(name="psA1", bufs=1, space="PSUM") as psA, \
             tc.tile_pool(name="ps_t", bufs=1, space="PSUM") as pst:

            idenb = cst.tile([128, 128], BF)
            make_identity(nc, idenb)

            h2_writes = []
            pctx = {}
            for part in parts_sel:
                p = parts[part]
                w1sb = cst.tile([FXD, 1300], BF, tag=f"w1sb{part}")
                nc.sync.dma_start(w1sb, p["w1ext"])
                w2sb = cst.tile([128, 1300], BF, tag=f"w2sb{part}")
                nc.sync.dma_start(w2sb, p["w2pack"])
                b1sb = cst.tile([128, HC], BF, tag=f"b1sb{part}")
                nc.sync.dma_start(b1sb, p["b1rep"])

                # ---- phase A ----
                arall = cst.tile([128, NWIN, H1], BF, tag=f"arall{part}")
                for c in range(NWIN):
                    xT = wp.tile([FXD, 128], BF, tag="xT")
                    nc.sync.dma_start(xT, p["xT"][:, 128 * c:128 * (c + 1)])
                    acc = ps.tile([128, 1300], F32, tag="acc",
                                  padded_shape=[128, 1536])
                    for n0, n1 in ((0, 512), (512, 1024), (1024, 1300)):
                        nc.tensor.matmul(acc[:, n0:n1], lhsT=xT,
                                         rhs=w1sb[:, n0:n1],
                                         start=True, stop=True)
                    hrow = wp.tile([128, HROW], BF, tag="hrow")
                    if dbg:
                        nc.vector.memset(hrow[:, 1290:HROW], 0.0)
                    nc.scalar.activation(hrow[:, 0:1290], acc[:, 0:1290],
                                         ACTF.Copy)
                    adrow = wp.tile([128, 128], BF, tag="adrow")
                    if dbg:
                        nc.vector.memset(adrow[:, 10:128], 0.0)
                    nc.vector.tensor_copy(adrow[:, 0:10], acc[:, 1290:1300])
                    nc.vector.tensor_copy(arall[:, c, :], acc[:, 1290:1300])
                    nc.sync.dma_start(p["H"][128 * c:128 * (c + 1), :], hrow)
                    nc.sync.dma_start(p["ADt"][128 * c:128 * (c + 1), :], adrow)

                arq = None
                if part == "q":
                    # q's dst windows are a core-dependent global slice of
                    # ADt; fetch their ar rows via one host-indexed gather.
                    qwi = cst.tile([128, (QWIN * 128) // 16], I16, tag="qwi")
                    nc.sync.dma_start(qwi, p["qwidx"])
                    arq = cst.tile([128, QWIN, 128], BF, tag="arq")
                    nc.gpsimd.dma_gather(arq, p["ADt"], qwi, QWIN * 128,
                                         QWIN * 128, 128, single_packet=False,
                                         queue_num=nextq())
                pctx[part] = (w1sb, w2sb, b1sb, arall, arq)

            # ---- GAT1 windows, f/q interleaved 4:1 to fill pipeline ----
            for part, w in _sched(parts_sel, max_win):
                    p = parts[part]
                    w1sb, w2sb, b1sb, arall, arq = pctx[part]
                    T = p["T"][w]
                    c0 = int(p["cumT"][w])
                    acc = ps.tile([128, 1426], F32, tag="acc",
                                  padded_shape=[128, 1536])
                    arl = arall[:, w, :] if part == "f" else arq[:, w, 0:10]
                    selftile = part == "f"
                    if selftile:
                        # self-loops: window's own H rows, contiguous load
                        # (no gather / one-hot tables); scatter = identity.
                        Gs = wp.tile([128, HROW], BF, tag="Gs")
                        nc.sync.dma_start(Gs, p["H"][128 * w:128 * (w + 1), :])
                        zs = wp.tile([128, H1], F32, tag="zs")
                        nc.vector.tensor_tensor(zs, Gs[:, 1280:1290], arl,
                                                op=ALU.add)
                        zs2 = wp.tile([128, H1], F32, tag="zs2")
                        nc.vector.tensor_scalar(zs2, zs, 0.2, None,
                                                op0=ALU.mult)
                        nc.vector.tensor_tensor(zs, zs, zs2, op=ALU.max)
                        pes = wp.tile([128, H1], BF, tag="pes")
                        nc.scalar.activation(pes, zs, ACTF.Exp)
                        Mcs = wp.tile([128, 1290], BF, tag="Mcs")
                        nc.vector.tensor_tensor(
                            Mcs[:, 0:1280].rearrange("p (d h) -> p d h", h=H1),
                            Gs[:, 0:1280].rearrange("p (d h) -> p d h", h=H1),
                            pes.rearrange("p (o h) -> p o h", o=1)
                               .to_broadcast([128, D, H1]),
                            op=ALU.mult)
                        nc.vector.tensor_copy(Mcs[:, 1280:1290], pes)
                        for n0, n1 in ((0, 512), (512, 1024), (1024, 1290)):
                            nc.tensor.matmul(acc[:, n0:n1], lhsT=idenb,
                                             rhs=Mcs[:, n0:n1],
                                             start=True, stop=False)
                    for t0, nt in chunks_of(T):
                        cc = c0 + t0
                        sidx = wp.tile([128, 8 * nt], I16, tag="sidx", bufs=4)
                        nc.sync.dma_start(sidx,
                                          p["esrc"][:, 8 * cc:8 * (cc + nt)])
                        G = wp.tile([128, nt, HROW], BF, tag="G", bufs=4)
                        nc.gpsimd.dma_gather(G, p["H"], sidx, nt * 128,
                                             nt * 128, HROW, single_packet=False,
                                             queue_num=nextq())
                        sTc = wp.tile([128, nt, 128], F8, tag="sTc", bufs=3)
                        nc.sync.dma_start(
                            sTc, p["stbl"][:, 128 * cc:128 * (cc + nt)])
                        sSc = wp.tile([128, nt, 128], F8, tag="sSc", bufs=3)
                        nc.sync.dma_start(
                            sSc, p["sstbl"][:, 128 * cc:128 * (cc + nt)])
                        adps = psA.tile([128, nt, H1], F32, tag="adps")
                        for tl in range(nt):
                            nc.tensor.matmul(adps[:, tl, :],
                                             lhsT=sTc[:, tl, :], rhs=arl,
                                             start=True, stop=True)
                        z = wp.tile([128, nt, H1], F32, tag="z")
                        nc.vector.tensor_tensor(z, G[:, :, 1280:1290],
                                                adps, op=ALU.add)
                        z2 = wp.tile([128, nt, H1], F32, tag="z2")
                        nc.vector.tensor_scalar(z2, z, 0.2, None, op0=ALU.mult)
                        nc.vector.tensor_tensor(z, z, z2, op=ALU.max)
                        pe = wp.tile([128, nt, H1], BF, tag="pe")
                        nc.scalar.activation(pe, z, ACTF.Exp)
                        # head-interleaved layout: pe broadcast has a real
                        # unit-stride innermost dim of 10 -> DVE 2x mode
                        Mc = wp.tile([128, nt, 1290], BF, tag="Mc")
                        nc.vector.tensor_tensor(
                            Mc[:, :, 0:1280].rearrange(
                                "p t (d h) -> p t d h", h=H1),
                            G[:, :, 0:1280].rearrange(
                                "p t (d h) -> p t d h", h=H1),
                            pe.rearrange("p t (o h) -> p t o h", o=1)
                              .to_broadcast([128, nt, D, H1]),
                            op=ALU.mult)
                        nc.vector.tensor_copy(Mc[:, :, 1280:1290], pe)
                        for tl in range(nt):
                            t = t0 + tl
                            for n0, n1 in ((0, 512), (512, 1024), (1024, 1290)):
                                nc.tensor.matmul(acc[:, n0:n1],
                                                 lhsT=sSc[:, tl, :],
                                                 rhs=Mc[:, tl, n0:n1],
                                                 start=(t == 0
                                                        and not selftile),
                                                 stop=(t == T - 1))
                    # window tail
                    rden = wp.tile([128, H1, 1], F32, tag="rden")
                    nc.vector.tensor_scalar(rden[:, :, 0], acc[:, 1280:1290],
                                            1e-30, None, op0=ALU.add)
                    nc.vector.reciprocal(rden[:, :, 0], rden[:, :, 0])
                    z1 = wp.tile([128, HC], BF, tag="z1")
                    nc.vector.tensor_tensor(
                        z1.rearrange("p (d h) -> p d h", h=H1),
                        acc[:, 0:1280].rearrange("p (d h) -> p d h", h=H1),
                        rden.rearrange("p h o -> p o h")
                            .to_broadcast([128, D, H1]),
                        op=ALU.mult)
                    nc.vector.tensor_tensor(z1, z1, b1sb, op=ALU.add)
                    e1 = wp.tile([128, HC], BF, tag="e1")
                    nc.vector.tensor_scalar(e1, z1, 0.0, None, op0=ALU.min)
                    nc.scalar.activation(e1, e1, ACTF.Exp)
                    r1 = wp.tile([128, HC], BF, tag="r1")
                    nc.scalar.activation(r1, z1, ACTF.Relu)
                    nc.vector.tensor_tensor(r1, r1, e1, op=ALU.add)
                    nc.vector.tensor_scalar(r1, r1, -1.0, None, op0=ALU.add)
                    # transposes + mm2 -> h2 (accumulate in acc's pad region)
                    h2ps = acc[:, 1296:1426]
                    for c in range(10):
                        tp = pst.tile([128, 128], BF, tag="tp")
                        nc.tensor.transpose(tp, r1[:, 128 * c:128 * (c + 1)],
                                            idenb)
                        o1T = mp.tile([128, 128], BF, tag="o1T")
                        nc.vector.tensor_copy(o1T, tp)
                        nc.tensor.matmul(h2ps, lhsT=o1T,
                                         rhs=w2sb[:, 130 * c:130 * (c + 1)],
                                         start=(c == 0), stop=(c == 9))
                    h2sb = wp.tile([128, H2ROW], BF, tag="h2sb")
                    if dbg:
                        nc.vector.memset(h2sb[:, 131:H2ROW], 0.0)
                    nc.vector.tensor_copy(h2sb[:, 0:128], h2ps[:, 0:128])
                    nc.vector.memset(h2sb[:, 128:129], 1.0)
                    nc.vector.tensor_copy(h2sb[:, 129:131], h2ps[:, 128:130])
                    wrh2 = nc.sync.dma_start(
                        p["H2w"][128 * w:128 * (w + 1), :], h2sb)
                    h2_writes.append(wrh2)

            flag = cst.tile([1, 1], F32)
            fence = nc.gpsimd.memset(flag, 0.0)
            for wrh2 in h2_writes:
                add_dep_helper(fence.ins, wrh2.ins, sync=True,
                               reason="fence H2 writes")

    if stage < 2:
        nc.compile()
        return nc
    if "q" in parts_sel:
        nc.gpsimd.collective_compute(
            "AllGather", mybir.AluOpType.bypass,
            replica_groups=[[0, 1, 2, 3], [4, 5, 6, 7]],
            ins=[parts["q"]["H2w"]], outs=[H2q_full],
        ).then_inc(sem1)
        nc.gpsimd.wait_ge(sem1, 1)

    # =======================================================================
    # TC2: layer-2 windows + pooling; full-branch Wg + contributions
    # =======================================================================
    with tile.TileContext(nc) as tc:
        with tc.tile_pool(name="const2", bufs=1) as cst, \
             tc.tile_pool(name="work2", bufs=2) as wp, \
             tc.tile_pool(name="mwork2", bufs=3) as mp, \
             tc.tile_pool(name="ps2", bufs=2, space="PSUM") as ps, \
             tc.tile_pool(name="psA2", bufs=1, space="PSUM") as psA, \
             tc.tile_pool(name="ps2t", bufs=2, space="PSUM") as pst:

            idenf = cst.tile([128, 128], F32, tag="idenf")
            make_identity(nc, idenf)
            idenb2 = cst.tile([128, 128], BF, tag="idenb2")
            make_identity(nc, idenb2)
            x11T = cst.tile([128, 512], F32, tag="x11T")
            nc.vector.memset(x11T, 0.0)
            x22T = cst.tile([128, 512], F32, tag="x22T")
            nc.vector.memset(x22T, 0.0)

            fence_deps = []
            pctx2 = {}
            for part in parts_sel:
                p = parts[part]
                b2sb = cst.tile([128, D], BF, tag=f"b2sb{part}")
                nc.sync.dma_start(b2sb, p["b2rep"])
                sent = cst.tile([1, H2ROW], BF, tag=f"sent{part}")
                nc.vector.memset(sent, NEG)
                nc.sync.dma_start(
                    p["OUT2"][p["sentinel"]:p["sentinel"] + 1, :], sent)
                pctx2[part] = b2sb

            # ---- GAT2 windows, f/q interleaved ----
            for part, w in _sched(parts_sel, max_win):
                    p = parts[part]
                    b2sb = pctx2[part]
                    T = p["T"][w]
                    c0 = int(p["cumT"][w])
                    ar2l = wp.tile([128, 1], BF, tag="ar2l")
                    nc.sync.dma_start(
                        ar2l, p["H2w"][128 * w:128 * (w + 1), 130:131])
                    acc2 = ps.tile([128, 129], F32, tag="acc2")
                    selftile = part == "f"
                    if selftile:
                        G2s = wp.tile([128, 131], BF, tag="G2s")
                        nc.sync.dma_start(
                            G2s, p["H2w"][128 * w:128 * (w + 1), 0:131])
                        z2s = wp.tile([128, 1], F32, tag="z2s")
                        nc.vector.tensor_tensor(z2s, G2s[:, 129:130], ar2l,
                                                op=ALU.add)
                        z2s2 = wp.tile([128, 1], F32, tag="z2s2")
                        nc.vector.tensor_scalar(z2s2, z2s, 0.2, None,
                                                op0=ALU.mult)
                        nc.vector.tensor_tensor(z2s, z2s, z2s2, op=ALU.max)
                        pe2s = wp.tile([128, 1], F32, tag="pe2s")
                        nc.scalar.activation(pe2s, z2s, ACTF.Exp)
                        M2s = wp.tile([128, 129], BF, tag="M2s")
                        nc.vector.tensor_tensor(
                            M2s, G2s[:, 0:129],
                            pe2s.to_broadcast([128, 129]), op=ALU.mult)
                        nc.tensor.matmul(acc2[:, 0:129], lhsT=idenb2,
                                         rhs=M2s, start=True, stop=False)
                    for t0, nt in chunks_of(T):
                        cc = c0 + t0
                        sidx = wp.tile([128, 8 * nt], I16, tag="sidx", bufs=4)
                        nc.sync.dma_start(sidx,
                                          p["esrc"][:, 8 * cc:8 * (cc + nt)])
                        G2 = wp.tile([128, nt, H2ROW], BF, tag="G2", bufs=4)
                        nc.gpsimd.dma_gather(G2, p["H2read"], sidx, nt * 128,
                                             nt * 128, H2ROW, single_packet=False,
                                             queue_num=nextq())
                        sTc = wp.tile([128, nt, 128], F8, tag="sTc", bufs=3)
                        nc.sync.dma_start(
                            sTc, p["stbl"][:, 128 * cc:128 * (cc + nt)])
                        sSc = wp.tile([128, nt, 128], F8, tag="sSc", bufs=3)
                        nc.sync.dma_start(
                            sSc, p["sstbl"][:, 128 * cc:128 * (cc + nt)])
                        ar2ps = psA.tile([128, nt, 1], F32, tag="ar2ps")
                        for tl in range(nt):
                            nc.tensor.matmul(ar2ps[:, tl, :],
                                             lhsT=sTc[:, tl, :], rhs=ar2l,
                                             start=True, stop=True)
                        z = wp.tile([128, nt, 1], F32, tag="z")
                        nc.vector.tensor_tensor(z, G2[:, :, 129:130],
                                                ar2ps, op=ALU.add)
                        z2 = wp.tile([128, nt, 1], F32, tag="z2")
                        nc.vector.tensor_scalar(z2, z, 0.2, None, op0=ALU.mult)
                        nc.vector.tensor_tensor(z, z, z2, op=ALU.max)
                        pe = wp.tile([128, nt, 1], F32, tag="pe")
                        nc.scalar.activation(pe, z, ACTF.Exp)
                        S2c = wp.tile([128, nt, 128], BF, tag="S2c")
                        nc.vector.tensor_tensor(
                            S2c, sSc, pe.to_broadcast([128, nt, 128]),
                            op=ALU.mult)
                        for tl in range(nt):
                            t = t0 + tl
                            nc.tensor.matmul(acc2[:, 0:129], lhsT=S2c[:, tl, :],
                                             rhs=G2[:, tl, 0:129],
                                             start=(t == 0 and not selftile),
                                             stop=(t == T - 1))
                    rden = wp.tile([128, 1], F32, tag="rden")
                    nc.vector.tensor_scalar(rden, acc2[:, 128:129], 1e-30,
                                            None, op0=ALU.add)
                    nc.vector.reciprocal(rden, rden)
                    z1 = wp.tile([128, D], BF, tag="z1")
                    nc.vector.tensor_tensor(z1, acc2[:, 0:128],
                                            rden.to_broadcast([128, D]),
                                            op=ALU.mult)
                    nc.vector.tensor_tensor(z1, z1, b2sb, op=ALU.add)
                    e1 = wp.tile([128, D], BF, tag="e1")
                    nc.vector.tensor_scalar(e1, z1, 0.0, None, op0=ALU.min)
                    nc.scalar.activation(e1, e1, ACTF.Exp)
                    o2 = wp.tile([128, H2ROW], BF, tag="o2")
                    if dbg:
                        nc.vector.memset(o2[:, 128:H2ROW], 0.0)
                    nc.scalar.activation(o2[:, 0:128], z1, ACTF.Relu)
                    nc.vector.tensor_tensor(o2[:, 0:128], o2[:, 0:128], e1,
                                            op=ALU.add)
                    nc.vector.tensor_scalar(o2[:, 0:128], o2[:, 0:128], -1.0,
                                            None, op0=ALU.add)
                    nc.sync.dma_start(p["OUT2"][128 * w:128 * (w + 1), :], o2)

            for part in parts_sel:
                p = parts[part]
                # ---- pooling ----
                pidx = wp.tile([128, (B * SLOT) // 16], I16, tag="pidx")
                nc.sync.dma_start(pidx, p["pidx"])
                Pg = wp.tile([128, SLOT * 4, H2ROW], BF, tag="Pg", bufs=1)
                # split pooling gather 4 ways across SWDGE queues
                npq = (B * SLOT) // NQ
                assert npq % 128 == 0 and (SLOT * 4) % NQ == 0
                for qq in range(NQ):
                    d1 = (SLOT * 4) // NQ
                    nc.gpsimd.dma_gather(
                        Pg[:, d1 * qq:d1 * (qq + 1), :], p["OUT2"],
                        pidx[:, (npq // 16) * qq:(npq // 16) * (qq + 1)],
                        npq, npq, H2ROW, single_packet=False, queue_num=qq)
                pooled = wp.tile([128, 4, H2ROW], F32, tag="pooled", bufs=1)
                nc.vector.tensor_reduce(
                    pooled,
                    Pg.rearrange("p (s gb) e -> p gb e s", s=SLOT),
                    op=ALU.max, axis=AX.X)

                if part == "f":
                    wg = cst.tile([128, 128], BF, tag="wg")
                    nc.sync.dma_start(wg, p["wgT"])
                    bgc = cst.tile([128, 1], F32, tag="bgcs")
                    nc.sync.dma_start(bgc, p["bgc"])
                    wsc = cst.tile([128, 2], F32, tag="wscs")
                    nc.sync.dma_start(wsc, p["wsc"])
                    gps = ps.tile([128, 512], F32, tag="gps")
                    for gb in range(4):
                        tp = pst.tile([128, 128], F32, tag="tp2")
                        nc.tensor.transpose(tp, pooled[:, gb, 0:128], idenf)
                        pT = mp.tile([128, 128], BF, tag="pT")
                        nc.vector.tensor_copy(pT, tp)
                        nc.tensor.matmul(gps[:, 128 * gb:128 * (gb + 1)],
                                         lhsT=wg, rhs=pT,
                                         start=True, stop=True)
                    gt = wp.tile([128, 512], F32, tag="gt", bufs=1)
                    nc.vector.tensor_scalar(gt, gps, bgc, 0.0, op0=ALU.add,
                                            op1=ALU.max)
                    c11 = wp.tile([128, 512], F32, tag="c11", bufs=1)
                    nc.vector.tensor_scalar(c11, gt, wsc[:, 0:1], None,
                                            op0=ALU.mult)
                    nc.vector.tensor_tensor(x11T, x11T, c11, op=ALU.add)
                    nc.vector.tensor_scalar(c11, gt, wsc[:, 1:2], None,
                                            op0=ALU.mult)
                    nc.vector.tensor_tensor(x22T, x22T, c11, op=ALU.add)
                else:
                    wrp = nc.sync.dma_start(
                        pooledq_in, pooled.rearrange("p gb e -> p (gb e)"))
                    fence_deps.append(wrp)

            wc1 = nc.sync.dma_start(C_accum[0:128, :], x11T)
            wc2 = nc.sync.dma_start(C_accum[128:256, :], x22T)
            fence_deps += [wc1, wc2]
            flag = cst.tile([1, 1], F32, tag="flag2")
            fence = nc.gpsimd.memset(flag, 0.0)
            for dpi in fence_deps:
                add_dep_helper(fence.ins, dpi.ins, sync=True, reason="fence2")

    if stage < 3:
        nc.compile()
        return nc
    nc.gpsimd.collective_compute(
        "AllReduce", mybir.AluOpType.max,
        replica_groups=[[0, 1, 2, 3], [4, 5, 6, 7]],
        ins=[pooledq_in], outs=[pooledq_red],
    ).then_inc(sem2)
    nc.gpsimd.wait_ge(sem2, 1)

    # =======================================================================
    # TC3: quarter-branch Wg + add contribution
    # =======================================================================
    with tile.TileContext(nc) as tc:
        with tc.tile_pool(name="const3", bufs=1) as cst, \
             tc.tile_pool(name="work3", bufs=2) as wp, \
             tc.tile_pool(name="ps3", bufs=1, space="PSUM") as ps, \
             tc.tile_pool(name="ps3t", bufs=2, space="PSUM") as pst:
            p = parts["q"]
            idenf = cst.tile([128, 128], F32)
            make_identity(nc, idenf)
            pq = cst.tile([128, 1024], F32)
            nc.gpsimd.dma_start(pq, pooledq_red)
            ca = cst.tile([128, 512], F32)
            nc.gpsimd.dma_start(ca, C_accum[0:128, :])
            cb = cst.tile([128, 512], F32)
            nc.gpsimd.dma_start(cb, C_accum[128:256, :])
            wg = cst.tile([128, 128], BF)
            nc.sync.dma_start(wg, p["wgT"])
            bgc = cst.tile([128, 1], F32)
            nc.sync.dma_start(bgc, p["bgc"])
            wsc = cst.tile([128, 2], F32)
            nc.sync.dma_start(wsc, p["wsc"])
            pooled = pq.rearrange("p (gb e) -> p gb e", gb=4)
            gps = ps.tile([128, 512], F32)
            for gb in range(4):
                tp = pst.tile([128, 128], F32, tag="tp3")
                nc.tensor.transpose(tp, pooled[:, gb, 0:128], idenf)
                pT = wp.tile([128, 128], BF, tag="pT3")
                nc.vector.tensor_copy(pT, tp)
                nc.tensor.matmul(gps[:, 128 * gb:128 * (gb + 1)], lhsT=wg,
                                 rhs=pT, start=True, stop=True)
            gt = cst.tile([128, 512], F32, tag="gt3")
            nc.vector.tensor_scalar(gt, gps, bgc, 0.0, op0=ALU.add,
                                    op1=ALU.max)
            cq = cst.tile([128, 512], F32, tag="cq")
            nc.vector.tensor_scalar(cq, gt, wsc[:, 0:1], None, op0=ALU.mult)
            nc.vector.tensor_tensor(ca, ca, cq, op=ALU.add)
            nc.vector.tensor_scalar(cq, gt, wsc[:, 1:2], None, op0=ALU.mult)
            nc.vector.tensor_tensor(cb, cb, cq, op=ALU.add)
            w1 = nc.sync.dma_start(C_in[0:128, :], ca)
            w2 = nc.sync.dma_start(C_in[128:256, :], cb)
            flag = cst.tile([1, 1], F32, tag="flag3")
            fence = nc.gpsimd.memset(flag, 0.0)
            add_dep_helper(fence.ins, w1.ins, sync=True, reason="fence3a")
            add_dep_helper(fence.ins, w2.ins, sync=True, reason="fence3b")

    nc.gpsimd.collective_compute(
        "AllReduce", mybir.AluOpType.add,
        replica_groups=[list(range(NCORE))],
        ins=[C_in], outs=[C_red],
    ).then_inc(sem3)
    nc.gpsimd.wait_ge(sem3, 1)

    # =======================================================================
    # TC4: dense tail (full batch on every core)
    # =======================================================================
    with tile.TileContext(nc) as tc:
        with tc.tile_pool(name="const4", bufs=1) as cst, \
             tc.tile_pool(name="work4", bufs=2) as wp, \
             tc.tile_pool(name="twp", bufs=2) as twp, \
             tc.tile_pool(name="ps4", bufs=2, space="PSUM") as ps, \
             tc.tile_pool(name="ps4t", bufs=2, space="PSUM") as pst:

            idenf = cst.tile([128, 128], F32)
            make_identity(nc, idenf)
            x11T = cst.tile([128, 512], F32)
            nc.gpsimd.dma_start(x11T, C_red[0:128, :])
            x22T = cst.tile([128, 512], F32)
            nc.gpsimd.dma_start(x22T, C_red[128:256, :])

            cn_chunks = []
            for r in range(4):
                cr = wp.tile([128, FXT], F32, tag="cellr")
                nc.sync.dma_start(cr, cell_in[128 * r:128 * (r + 1), :])
                sq = wp.tile([128, FXT], F32, tag="sqc")
                nc.vector.tensor_tensor(sq, cr, cr, op=ALU.mult)
                rn = wp.tile([128, 1], F32, tag="rn")
                nc.vector.tensor_reduce(rn, sq, op=ALU.add, axis=AX.X)
                nc.scalar.activation(rn, rn, ACTF.Sqrt)
                nc.vector.reciprocal(rn, rn)
                cn = cst.tile([128, 1024], F32, tag=f"cn{r}")
                nc.vector.tensor_scalar(cn[:, 0:FXT], cr, rn, None,
                                        op0=ALU.mult)
                nc.vector.memset(cn[:, FXT:1024], 0.0)
                cn_chunks.append(cn)
            cT = []
            for k in range(8):
                ct = cst.tile([128, 512], F32, tag=f"cT{k}")
                for r in range(4):
                    tp = pst.tile([128, 128], F32, tag="tp4")
                    nc.tensor.transpose(
                        tp, cn_chunks[r][:, 128 * k:128 * (k + 1)], idenf)
                    nc.vector.tensor_copy(ct[:, 128 * r:128 * (r + 1)], tp)
                cT.append(ct)

            def dense(chunks_in, win, bin_, kdim, mdim, tagp):
                nk = kdim // 128
                nm = mdim // 128
                assert len(chunks_in) == nk
                bsb = cst.tile([128, nm], F32, tag=f"b{tagp}")
                nc.sync.dma_start(bsb, bin_)
                outs = []
                for m in range(nm):
                    wsb = twp.tile([128, nk, 128], F32, tag="tw")
                    nc.sync.dma_start(
                        wsb,
                        win.rearrange("(k p) m -> p k m", p=128)
                           [:, :, 128 * m:128 * (m + 1)])
                    acc = ps.tile([128, 512], F32, tag="acc4")
                    for k in range(nk):
                        nc.tensor.matmul(acc, lhsT=wsb[:, k, :],
                                         rhs=chunks_in[k],
                                         start=(k == 0), stop=(k == nk - 1))
                    ot = cst.tile([128, 512], F32, tag=f"o{tagp}{m}")
                    nc.vector.tensor_scalar(ot, acc, bsb[:, m:m + 1], 0.0,
                                            op0=ALU.add, op1=ALU.max)
                    outs.append(ot)
                return outs

            r1 = dense(cT, tw["wr1T"], tw["br1c"], 1024, 2048, "r1")
            r2 = dense(r1, tw["wr2T"], tw["br2c"], 2048, 512, "r2")
            r3 = dense(r2, tw["wr3T"], tw["br3c"], 512, 256, "r3")

            xc = [x11T, x22T, r3[0], r3[1]]
            ones = cst.tile([128, 1], F32, tag="ones")
            nc.vector.memset(ones, 1.0)
            n2ps = ps.tile([1, 512], F32, tag="n2ps", bufs=1)
            for i, chk in enumerate(xc):
                sq = wp.tile([128, 512], F32, tag="sq")
                nc.vector.tensor_tensor(sq, chk, chk, op=ALU.mult)
                nc.tensor.matmul(n2ps, lhsT=ones, rhs=sq,
                                 start=(i == 0), stop=(i == 3))
            nrm = wp.tile([1, 512], F32, tag="nrm4")
            nc.scalar.activation(nrm, n2ps, ACTF.Sqrt)
            rn4 = wp.tile([1, 512], F32, tag="rn4")
            nc.vector.reciprocal(rn4, nrm)
            rnb = cst.tile([128, 512], F32, tag="rnb")
            nc.gpsimd.partition_broadcast(rnb, rn4)
            xcn = []
            for i, chk in enumerate(xc):
                o = cst.tile([128, 512], F32, tag=f"xcn{i}")
                nc.vector.tensor_tensor(o, chk, rnb, op=ALU.mult)
                xcn.append(o)

            f1 = dense(xcn, tw["wf1T"], tw["bf1c"], 512, 2048, "f1")
            f2 = dense(f1, tw["wf2T"], tw["bf2c"], 2048, 512, "f2")
            f3 = dense(f2, tw["wf3T"], tw["bf3c"], 512, 128, "f3")

            wo = cst.tile([128, 2], F32, tag="wo")
            nc.sync.dma_start(wo, tw["woT"])
            bo = cst.tile([2, 1], F32, tag="bo")
            nc.sync.dma_start(bo, tw["boc"])
            ops_ = ps.tile([2, 512], F32, tag="ops", bufs=1)
            nc.tensor.matmul(ops_, lhsT=wo, rhs=f3[0], start=True, stop=True)
            osb = cst.tile([2, 512], F32, tag="osb")
            nc.vector.tensor_scalar(osb, ops_, bo, None, op0=ALU.add)
            nc.sync.dma_start(out_ext, osb)

    nc.compile()
    return nc


_CACHE = {}


def kernel(**inputs) -> np.ndarray:
    from concourse.bass_utils import run_bass_kernel_spmd

    in_maps, consts = _prep_inputs(inputs)
    key = (tuple(consts["T_full"]), tuple(consts["T_qtr"]), consts["slot"])
    if key not in _CACHE:
        _CACHE[key] = _build(consts)
    nc = _CACHE[key]
    res = run_bass_kernel_spmd(nc, in_maps, core_ids=list(range(NCORE)))
    return np.ascontiguousarray(
        np.asarray(res.results[0]["out"]).T).astype(np.float32)

